# revision 39
# baseline (speedup 1.0000x reference)
"""Trainium2 Bass kernel for nn_DecentLayer (gnn_message_passing).

The reference gathers 16 of 24 input channels via static position matching,
then runs a 3x3 same-padded conv: [B=16, 16, 256, 256] x [32, 16, 3, 3]
-> [B, 32, 256, 256].

Strategy (v12): row-shift K packing (v3 math) + measured schedule fixes.
  * Data-parallel over batch: 8 cores x 2 images.
  * K = 96 partitions = 6 row-shifts (j) x 16 channels. Partition (j, ch)
    holds the padded rows 32s + 4m' + j (m' = 0..7) of each 32-row strip s,
    so for a 4-row output block at base h (h = 4m'), ALL THREE dh taps of
    output rows h..h+3 read the same per-partition row index m'.
    Data replication is 1.5x (6 j-copies covering stride-4 rows).
  * M = 128 = 4 row-residues (rr) x 32 filters. Stationary w[(j,ch),(rr,f)]
    = W[f, ch, dh=j-rr, dw] for 0 <= j-rr <= 2 (banded), else 0.
  * One matmul per dw in {0,1,2} with N = 512 (2 row-blocks x 256 cols)
    covers all 9 taps for 8 output rows. PSUM [128, 512] f32 = one bank.
  * Schedule facts this build is tuned around (all measured on trn2):
      - Any DMA trigger costs ~0.65 us engine time and ~0.75 us
        retire-to-first-packet latency; completion sems lag the last
        packet by ~0.5 us; all rings share the 16 DMA engines. So: few,
        well-placed triggers, and critical loads never behind a flood.
      - The PE p-state ramps over ~3 us of continuous work and resets on
        idle gaps. 32 N=64 warmup matmuls run before the first real
        matmul, and "bridge" warmup bursts cover the two known early
        load waits (x000c after strip0 q1, strip 1 after q3).
      - Weights load on the scalar HWDGE ring in parallel with x loads
        on sync; first strip split a/b/c so the PE starts after ~100 KB.
      - Stores: strips 0..14 via the gpsimd SWDGE ring (its trigger
        waits block nothing); the final strip stores q0..q2 as 128 KB
        chunks alternating sync/scalar and q3 split-cast across
        vector+scalar with two 64 KB half-stores, one per ring.
"""

import numpy as np
import ml_dtypes

import concourse.bass as bass
import concourse.bacc as bacc
import concourse.mybir as mybir
import concourse.tile as tile
from concourse.bass_utils import run_bass_kernel_spmd

# Problem constants (hardcoded per the harness contract).
N_CORES = 8
B = 16
IMGS_PER_CORE = B // N_CORES  # 2
CIN = 16      # conv input channels after gather
COUT = 32     # filters
H = W = 256
SLOTS = 8         # strips per image
HS = H // SLOTS   # 32 output rows per strip
WP = W + 2        # padded row width
NJ = 6            # row-shift copies (j = rr + dh, rr in 0..3, dh in 0..2)
KP = NJ * CIN     # 96 K partitions
NQ = 4            # psum tiles per strip (8 output rows each)
NDW = 3           # matmuls per psum tile (one per dw)
N_WARMUP = 48     # p-state ramp matmuls (N=64 each, ~58 ns apiece)


def _common_pairs(ms_in, ns_in, ms_x, ns_x):
    ms_in = np.asarray(ms_in)
    ns_in = np.asarray(ns_in)
    ms_x = np.asarray(ms_x)
    ns_x = np.asarray(ns_x)
    f_ids, x_ids = [], []
    for i_in in range(ms_in.shape[0]):
        hits = np.nonzero((ms_x == ms_in[i_in]) & (ns_x == ns_in[i_in]))[0]
        for i_x in hits:
            f_ids.append(i_in)
            x_ids.append(int(i_x))
    return np.asarray(f_ids), np.asarray(x_ids)


def build_program(n_img=IMGS_PER_CORE):
    f32 = mybir.dt.float32
    bf16 = mybir.dt.bfloat16

    nc = bacc.Bacc("TRN2", target_bir_lowering=False, debug=False)
    # x: per (img, strip): [96 parts, 2 halves, 4 rows, WP] contiguous.
    x_in = nc.dram_tensor("x", [n_img, SLOTS, KP, 2, 4, WP], bf16,
                          kind="ExternalInput")
    w_in = nc.dram_tensor("w", [KP, NDW, 128], bf16, kind="ExternalInput")
    # Permuted output layout: output row h = 32*s + 4*k + rr (k = 2*q + blk).
    # Stored as [b, s, rr, co, k, w] so each per-strip store is a fully
    # contiguous 512 KB block (4 KB per partition); host transposes back.
    y_out = nc.dram_tensor("y", [n_img, SLOTS, 4, COUT, 8, W], bf16,
                           kind="ExternalOutput")

    with tile.TileContext(nc) as tc:
        with (
            tc.tile_pool(name="persist", bufs=1) as persist,
            tc.tile_pool(name="op", bufs=4) as op,
            tc.tile_pool(name="ps", bufs=7, space="PSUM") as psp,
            tc.tile_pool(name="psw", bufs=1, space="PSUM") as pswp,
        ):
            # Weights in ONE DMA on the scalar HWDGE ring, in parallel with
            # the x loads on the sync ring (both rings share the 16 DMA
            # engines, but the critical bytes here are small).
            wtt = persist.tile([KP, NDW, 128], bf16, name="wt")
            nc.scalar.dma_start(out=wtt[:], in_=w_in[:])

            def wt(dw):
                return wtt[:, dw, :]

            # PE p-state warmup: the PE clock ramps to full speed over ~3 us
            # of continuous execution and resets on any idle gap. N=64
            # warmups run ~62 ns each; overshoot past data-arrival is cheap,
            # a gap (ramp reset) is not, so oversize the count.
            scr = persist.tile([KP, 256], bf16, name="scr")
            nc.vector.memset(scr[:], 0.0)
            ps_w = pswp.tile([128, 2 * W], f32, name="warm")
            for _ in range(N_WARMUP):
                nc.tensor.matmul(ps_w[:, :64], scr[:, :128], scr[:, 128:192],
                                 start=True, stop=True)

            # All x tiles resident (66 KB/partition on 96 partitions).
            # First strip split in three (each trigger costs ~0.65 us of
            # sync-engine time, so keep the early trigger count low while
            # still letting the PE start after ~100 KB).
            xbufs = {}
            xa = persist.tile([KP, 2, WP], bf16, name="x000a")
            xb = persist.tile([KP, 2, WP], bf16, name="x000b")
            xc = persist.tile([KP, 4, WP], bf16, name="x000c")
            nc.sync.dma_start(out=xa[:], in_=x_in[0, 0, :, 0, 0:2])
            nc.sync.dma_start(out=xb[:], in_=x_in[0, 0, :, 0, 2:4])
            # The early load phase is single-queue limited (~160 B/ns);
            # route xc and strips 2/4 through the scalar HWDGE ring (idle
            # after the wt load) so both queues' packets interleave on the
            # 16 engines and strip 1 lands ~4 us earlier on the sync ring.
            nc.scalar.dma_start(out=xc[:], in_=x_in[0, 0, :, 1])
            xbufs[0, 0] = (xa, xb, xc)
            for p in range(n_img):
                for s in range(SLOTS):
                    if p == 0 and s == 0:
                        continue
                    xt = persist.tile([KP, 2, 4, WP], bf16, name=f"x{p}s{s}")
                    ring = nc.scalar if p * SLOTS + s in (2, 4) else nc.sync
                    ring.dma_start(out=xt[:], in_=x_in[p, s])
                    xbufs[p, s] = (xt,)

            def xview(p, s, q):
                """Moving view [KP, 2, WP] for psum tile q (rows 2q..2q+1)."""
                seg = xbufs[p, s]
                hf, mq = divmod(q, 2)
                if len(seg) == 3:
                    if q == 0:
                        return seg[0][:]
                    if q == 1:
                        return seg[1][:]
                    return seg[2][:, 2 * mq: 2 * mq + 2]
                return seg[0][:, hf, 2 * mq: 2 * mq + 2]

            def bridge(n):
                """Keep the PE clock ramped across a known data-wait: a
                short burst of warmup matmuls instead of an idle gap (any
                idle gap resets the p-state ramp)."""
                for _ in range(n):
                    nc.tensor.matmul(ps_w[:, :64], scr[:, :128],
                                     scr[:, 128:192], start=True, stop=True)

            for p in range(n_img):
                for s in range(SLOTS):
                    idx = p * SLOTS + s
                    last = idx == n_img * SLOTS - 1
                    outt = op.tile([128, NQ * 2 * W], bf16, name="ot")
                    for q in range(NQ):
                        xv = xview(p, s, q)
                        ps = psp.tile([128, 2 * W], f32, name="acc")
                        for dw in range(NDW):
                            nc.tensor.matmul(
                                ps[:],
                                wt(dw),
                                xv[:, :, dw: dw + W],
                                start=(dw == 0),
                                stop=(dw == NDW - 1),
                            )
                        # Bridge the early load-pipeline waits (x000c after
                        # q1, strip 1 after q3) without idling the PE.
                        if idx == 0 and q == 1:
                            bridge(8)
                        elif idx == 0 and q == 3:
                            bridge(4)
                        dst = outt[:, q * 2 * W: (q + 1) * 2 * W]
                        if not last:
                            # Alternate cast engines; never attach DMA
                            # triggers to the cast engines mid-stream (a
                            # waiting trigger head-of-line blocks casts and
                            # starves the PE of psum banks).
                            if q % 2 == 0:
                                nc.vector.tensor_copy(dst, ps[:])
                            else:
                                nc.scalar.copy(dst, ps[:])
                        else:
                            # Final strip: per-ring triggers cost ~0.7 us
                            # each plus ~0.75 us gen latency, so keep at
                            # most 2-3 triggers per ring. q0..q2 are full
                            # 128 KB stores alternating rings; q3 is cast
                            # split across both engines and stored as two
                            # 64 KB halves, one per ring, so the last chunk
                            # has minimal cast+flow latency.
                            if q < NQ - 1:
                                if q % 2 == 0:
                                    nc.vector.tensor_copy(dst, ps[:])
                                    ring = nc.sync
                                else:
                                    nc.scalar.copy(dst, ps[:])
                                    ring = nc.scalar
                                ring.dma_start(
                                    out=y_out[p, s, :, :, 2 * q: 2 * q + 2],
                                    in_=dst,
                                )
                            else:
                                nc.vector.tensor_copy(dst[:, :W], ps[:, :W])
                                nc.scalar.copy(dst[:, W:], ps[:, W:])
                                nc.sync.dma_start(
                                    out=y_out[p, s, :, :, 2 * q: 2 * q + 1],
                                    in_=dst[:, :W],
                                )
                                nc.scalar.dma_start(
                                    out=y_out[p, s, :, :, 2 * q + 1: 2 * q + 2],
                                    in_=dst[:, W:],
                                )
                    # Whole-strip stores via the gpsimd SWDGE ring for all
                    # but the final strip (gpsimd is otherwise idle, and a
                    # waiting trigger there blocks nothing).
                    if not last:
                        nc.gpsimd.dma_start(out=y_out[p, s], in_=outt[:])

    nc.compile()
    return nc


_NC_CACHE = {}


def _get_program():
    if "v4" not in _NC_CACHE:
        _NC_CACHE["v4"] = build_program()
    return _NC_CACHE["v4"]


def _host_prep(inputs):
    x = np.asarray(inputs["x_data"], dtype=np.float32)
    w = np.asarray(inputs["weights"], dtype=np.float32)
    f_ids, x_ids = _common_pairs(
        inputs["ms_in"], inputs["ns_in"], inputs["ms_x"], inputs["ns_x"]
    )
    assert len(f_ids) == CIN, f"expected {CIN} matched pairs, got {len(f_ids)}"
    xg = x[:, x_ids]                                 # [B, 16, H, W]
    wg = w[:, f_ids]                                 # [COUT, 16, 3, 3]

    bf = ml_dtypes.bfloat16
    pad = np.zeros((B, CIN, H + 2, WP), dtype=bf)
    pad[:, :, 1: H + 1, 1: W + 1] = xg.astype(bf)

    # host_x[b, s, 16j+ch, m', c] = pad[b, ch, 32s + 4m' + j, c]
    host = np.empty((B, SLOTS, KP, 8, WP), dtype=bf)
    sm = 32 * np.arange(SLOTS)[:, None] + 4 * np.arange(8)[None, :]  # [s, m']
    for j in range(NJ):
        # pad[:, :, sm+j, :] -> [B, ch, s, m', c] -> [B, s, ch, m', c]
        host[:, :, CIN * j: CIN * (j + 1)] = pad[:, :, sm + j, :].transpose(
            0, 2, 1, 3, 4
        )
    host = host.reshape(B, SLOTS, KP, 2, 4, WP)

    # Stationaries [96, 3, 128]: w[(j,ch), dw, (rr,f)] = wg[f,ch,j-rr,dw]
    w_host = np.zeros((KP, NDW, 128), dtype=np.float32)
    for j in range(NJ):
        for rr in range(4):
            dh = j - rr
            if 0 <= dh <= 2:
                for dw in range(NDW):
                    w_host[CIN * j: CIN * (j + 1), dw,
                           32 * rr: 32 * rr + 32] = wg[:, :, dh, dw].T
    w_host = w_host.astype(bf)
    return host, w_host


def _run(inputs, trace=False):
    xh, w_host = _host_prep(inputs)
    nc = _get_program()
    in_maps = [
        {"x": xh[IMGS_PER_CORE * k: IMGS_PER_CORE * (k + 1)], "w": w_host}
        for k in range(N_CORES)
    ]
    res = run_bass_kernel_spmd(nc, in_maps, list(range(N_CORES)), trace=trace)
    # y stored as [n_img, s, rr, co, k, w]; h = 32*s + 4*k + rr
    outs = []
    for r in res.results:
        yp = np.asarray(r["y"]).astype(np.float32)
        outs.append(
            yp.transpose(0, 3, 1, 4, 2, 5).reshape(IMGS_PER_CORE, COUT, H, W)
        )
    out = np.concatenate(outs, axis=0)
    return out, res


def kernel(**inputs):
    out, _ = _run(inputs, trace=False)
    return out
